# revision 24
# baseline (speedup 1.0000x reference)
"""Trainium2 Bass kernel for nn_CAModel (neural cellular automaton step).

Per-core (8-way batch-parallel, 2 images/core) bf16 pipeline, v2.

Key structure (vs v1): the MLP is PHASE-BATCHED per (img, t)-window so the
PE never alternates tiling modes instruction-to-instruction (mode switches
drain the array and keep HAM throttled at K=4/8):
  - L1: 32x128 row-tiled mode, 4 dense K=32 MMs (4 strips, 4 banks)
    issued back-to-back -> they run CONCURRENTLY in the array (~2.9x),
    then 4 accumulating dwy MMs.
  - L2: full 128x128 mode, 8 back-to-back K=128 MMs (w2t stationary).
  - L3: 128x32 col-tiled mode, 8 MMs (4 col positions x 2 accum).
  - z1/z2 are [128,1024] 2-bank PSUM tiles evacuated in single N=1024
    relu+bias ops, split between ScalarE (ACT, 1.09ns/col) and VectorE
    (1.25ns/col) which are the critical path (~1x each; PSUM reads can't
    use 2x/4x DVE modes on TRN2).
  - depthwise sobel build: vertical passes on DVE, horizontal (odd-shift)
    on GPSIMD; startup reordered so window (img0,t0) starts ~10us in.
  - life masks via 5-row overlapped stripe maxpool (no cross-stripe halo
    DMA round trips); img0's epilogue runs under img1's MLP (GPSIMD),
    img1's epilogue is the tail (DVE).
"""

import numpy as np
import ml_dtypes
import concourse.bass as bass
import concourse.tile as tile
from concourse import bacc, mybir

AF = mybir.ActivationFunctionType
OP = mybir.AluOpType
f16 = mybir.dt.bfloat16
f32 = mybir.dt.float32

BL, C, H, W = 2, 16, 192, 192   # per-core images
U, RPU = 8, 24                  # row-block units per image, rows per unit
NT, TS = 9, 512                 # tiles per (img,u), pixels per tile
HID = 128
FHI = (RPU + 2) * W             # 4992 per img in halo'd layout
RC = 6                          # dw build chunk rows
WARMN = 12                      # PE warmup matmuls
DBG = False                     # extra debug outputs


def build_nc():
    nc = bacc.Bacc("TRN2", target_bir_lowering=False, debug=False)

    x_d = nc.dram_tensor("x", [BL, C, H, W], f16, kind="ExternalInput")
    fn_d = nc.dram_tensor("fn", [BL, H, W], f16, kind="ExternalInput")  # host umask {0,1}
    wstackA_d = nc.dram_tensor("wstackA", [128, 128], f16, kind="ExternalInput")
    w1y_d = nc.dram_tensor("w1y", [128, 256], f16, kind="ExternalInput")
    w2t_d = nc.dram_tensor("w2t", [128, 128], f16, kind="ExternalInput")
    w3t_d = nc.dram_tensor("w3t", [128, 64], f16, kind="ExternalInput")
    b1_d = nc.dram_tensor("b1", [128, 1], f32, kind="ExternalInput")
    b2_d = nc.dram_tensor("b2", [128, 1], f32, kind="ExternalInput")
    b3_d = nc.dram_tensor("b3", [128, 1], f32, kind="ExternalInput")
    out_d = nc.dram_tensor("out", [BL, C, H, W], f16, kind="ExternalOutput")

    with tile.TileContext(nc) as tc:
        with (
            tc.tile_pool(name="const", bufs=1) as const,
            tc.tile_pool(name="xf", bufs=1) as xfp,
            tc.tile_pool(name="chk", bufs=1) as chk,
            tc.tile_pool(name="chk2", bufs=2) as chk2,
            tc.tile_pool(name="msk", bufs=1) as mskp,
            tc.tile_pool(name="strp", bufs=1) as strp,
            tc.tile_pool(name="h1p", bufs=8) as h1p,
            tc.tile_pool(name="h2p", bufs=8) as h2p,
            tc.tile_pool(name="big", bufs=1) as bigp,
            tc.tile_pool(name="dram", bufs=1, space="DRAM") as dramp,
            tc.tile_pool(name="pz1", bufs=2, space="PSUM") as pz1,
            tc.tile_pool(name="pz2", bufs=2, space="PSUM") as pz2,
        ):
            # ---- constants (first: tiny, and warmup needs them) ----
            wstackA = const.tile([128, 128], f16)
            nc.sync.dma_start(wstackA[:], wstackA_d.ap())
            w1y = const.tile([128, 256], f16)
            nc.sync.dma_start(w1y[:], w1y_d.ap())
            w2t = const.tile([128, 128], f16)
            nc.sync.dma_start(w2t[:], w2t_d.ap())
            w3t = const.tile([128, 64], f16)
            nc.sync.dma_start(w3t[:], w3t_d.ap())
            b1c = const.tile([128, 1], f32)
            nc.sync.dma_start(b1c[:], b1_d.ap())
            b2c = const.tile([128, 1], f32)
            nc.sync.dma_start(b2c[:], b2_d.ap())
            b3c = const.tile([128, 1], f32)
            nc.sync.dma_start(b3c[:], b3_d.ap())

            # ---- xf: halo'd x, partition p = u*16 + c; buffer row r = img row
            # u*24 + r - 1.  Halo rows at image top/bottom stay zero (zero-pad
            # is equivalent to -inf pad for the >0.1 life test).
            xf = xfp.tile([128, BL, RPU + 2, W], f16)
            nc.vector.memset(xf[0:32, :, 0:1, :], 0.0)
            nc.vector.memset(xf[96:128, :, 25:26, :], 0.0)

            def load_xf(img):
                for u in range(U):
                    lo = max(0, u * RPU - 1)
                    hi = min(H, u * RPU + RPU + 1)
                    rb0 = 1 - (u * RPU - lo)
                    nc.sync.dma_start(
                        xf[u * 16:(u + 1) * 16, img, rb0:rb0 + (hi - lo), :],
                        x_d.ap()[img, :, lo:hi, :],
                    )

            # ---- P1/P2: [x(u) | dwx(u)] interleaved per u; P1 even u, P2 odd.
            p1t = xfp.tile([128, BL, RPU + 2, W], f16, name="p1t")
            p2t = xfp.tile([128, BL, RPU + 2, W], f16, name="p2t")
            nc.vector.memset(p1t[0:16, :, 0:1, :], 0.0)     # u=0 top halo (x half)
            nc.vector.memset(p2t[96:112, :, 25:26, :], 0.0)  # u=7 bottom halo

            def load_px(img):
                for u in range(U):
                    lo = max(0, u * RPU - 1)
                    hi = min(H, u * RPU + RPU + 1)
                    rb0 = 1 - (u * RPU - lo)
                    pt = p1t if u % 2 == 0 else p2t
                    pu = (u // 2) * 32
                    nc.sync.dma_start(
                        pt[pu:pu + 16, img, rb0:rb0 + (hi - lo), :],
                        x_d.ap()[img, :, lo:hi, :],
                    )

            dwys_t = [xfp.tile([128, RPU, W], f16, name=f"dwy{i}") for i in range(BL)]

            # ---- depthwise sobel build: verticals (aligned) on DVE,
            # horizontals (odd shifts) on GPSIMD; dwx DMA'd into P1/P2.
            def emit_dw_chunk(img, r0, he):
                flat = lambda ap: ap.rearrange("p a b -> p (a b)")
                ps = chk.tile([128, RC + 1, W], f16, tag="ps")
                nc.vector.tensor_add(
                    flat(ps[:]), flat(xf[:, img, r0:r0 + RC + 1, :]),
                    flat(xf[:, img, r0 + 1:r0 + RC + 2, :])
                )
                v1 = chk.tile([128, RC, W], f16, tag="v1")
                nc.vector.tensor_add(flat(v1[:]), flat(ps[:, 0:RC, :]), flat(ps[:, 1:RC + 1, :]))
                v2 = chk.tile([128, RC, W], f16, tag="v2")
                nc.vector.tensor_sub(
                    flat(v2[:]), flat(xf[:, img, r0 + 2:r0 + RC + 2, :]),
                    flat(xf[:, img, r0:r0 + RC, :])
                )
                qs = chk.tile([128, RC, W], f16, tag="qs")
                he.tensor_add(qs[:, :, 0:191], v2[:, :, 0:191], v2[:, :, 1:192])
                stg = chk2.tile([128, RC, W], f16, tag="dwxs", name=f"dwxs_{img}_{r0}")
                dxs = stg[:, :, :]
                dys = dwys_t[img][:, r0:r0 + RC, :]
                # dwx = v1[c+1] - v1[c-1]; borders zero-padded
                he.tensor_sub(dxs[:, :, 1:191], v1[:, :, 2:192], v1[:, :, 0:190])
                he.tensor_copy(dxs[:, :, 0:1], v1[:, :, 1:2])
                he.tensor_scalar_mul(dxs[:, :, 191:192], v1[:, :, 190:191], -1.0)
                # dwy = qs[c-1] + qs[c]; borders: qs[0]+v2[0], qs[190]+v2[191]
                he.tensor_add(dys[:, :, 1:191], qs[:, :, 0:190], qs[:, :, 1:191])
                he.tensor_add(dys[:, :, 0:1], qs[:, :, 0:1], v2[:, :, 0:1])
                he.tensor_add(dys[:, :, 191:192], qs[:, :, 190:191], v2[:, :, 191:192])
                for u in range(U):
                    pt = p1t if u % 2 == 0 else p2t
                    pu = (u // 2) * 32
                    nc.sync.dma_start(
                        pt[pu + 16:pu + 32, img, r0 + 1:r0 + 1 + RC, :],
                        stg[u * 16:(u + 1) * 16, :, :],
                    )

            # ---- update mask broadcast over channels ----
            umasks = [mskp.tile([128, RPU, W], f16, tag=f"um{i}", name=f"um{i}") for i in range(BL)]

            def load_um(img):
                for u in range(U):
                    src = fn_d.ap()[img, u * RPU:(u + 1) * RPU, :]
                    src = src.rearrange("a b -> (a b)").partition_broadcast(16)
                    nc.sync.dma_start(umasks[img][u * 16:(u + 1) * 16], src)

            # ================= startup emission =================
            load_xf(0)

            # PE warmup (full mode) while DMAs land
            zw = pz1.tile([128, 2 * TS], f32, tag="z1", name="zw")
            for i in range(WARMN):
                nc.tensor.matmul(zw[:, 0:256], w2t[:, :], w1y[:, 0:256],
                                 start=True, stop=True)

            load_px(0)
            for r0 in range(0, RPU, RC):
                emit_dw_chunk(0, r0, nc.vector if r0 <= RC else nc.gpsimd)
            load_um(0)

            # ---- 5-row overlapped stripe maxpool helpers ----
            # stripe layout: partition p = img*64 + u*8 + s; each stripe holds
            # band rows 3s-1 .. 3s+3 (5 rows, overlapped); output m2 = 3x3 max
            # over the 3 center rows, entirely stripe-local.
            xf_r = xf[:].rearrange("(u c) i r w -> u c i r w", c=16)

            def stripe_pool_chain(al5, sl, m2_out):
                """al5: [128,5,W] alpha stripes (5 rows); sl: partition slice;
                m2_out[sl] <- 3x192 maxpool rows."""
                pm = strp.tile([128, 5, 191], f16, tag="pm5")
                nc.vector.tensor_max(pm[sl], al5[sl, :, 0:191], al5[sl, :, 1:192])
                m1 = strp.tile([128, 5, W], f16, tag="m15")
                nc.vector.tensor_max(m1[sl, :, 1:191], pm[sl, :, 0:190], pm[sl, :, 1:191])
                nc.vector.tensor_copy(m1[sl, :, 0:1], pm[sl, :, 0:1])
                nc.vector.tensor_copy(m1[sl, :, 191:192], pm[sl, :, 190:191])
                t1 = strp.tile([128, 3, W], f16, tag="t15")
                nc.vector.tensor_max(t1[sl], m1[sl, 0:3, :], m1[sl, 1:4, :])
                nc.vector.tensor_max(m2_out[sl], t1[sl], m1[sl, 2:5, :])

            # pre-life: from xf alpha (xf buffer rows 3s..3s+4 = band rows
            # 3s-1..3s+3, halos included and zero at image edges).
            al5_pre = strp.tile([128, 5, W], f16, tag="al5pre")
            m2pre = strp.tile([128, 3, W], f16, tag="m2pre")

            def row_stripes(al5, img, rows2d, skip_memset=False):
                """al5[img*64+k, dr, :] <- rows2d[3k+dr-1, :] (global alpha rows;
                out-of-image rows left at memset 0 == zero-pad)."""
                if not skip_memset:
                    nc.gpsimd.memset(al5[img * 64:(img + 1) * 64, :, :], 0.0)
                for dr in range(5):
                    a = dr - 1
                    k0 = 0
                    if a < 0:
                        a += 3
                        k0 = 1
                    n = min(64 - k0, (H - a) // 3)
                    src = rows2d[a:a + 3 * n, :].rearrange(
                        "(k q) w -> k q w", q=3)[:, 0, :]
                    nc.sync.dma_start(
                        al5[img * 64 + k0:img * 64 + k0 + n, dr, :], src)
                    ktop = k0 + n
                    gtop = 3 * ktop + dr - 1
                    if ktop < 64 and gtop < H:
                        nc.sync.dma_start(
                            al5[img * 64 + ktop:img * 64 + ktop + 1, dr, :],
                            rows2d[gtop:gtop + 1, :])

            def emit_prelife(img):
                sl = slice(img * 64, (img + 1) * 64)
                row_stripes(al5_pre, img, x_d.ap()[img, 3, :, :])
                stripe_pool_chain(al5_pre, sl, m2pre)

            emit_prelife(0)

            # ================= MLP machinery =================
            p1_flat = p1t[:].rearrange("p i r w -> p (i r w)")
            p2_flat = p2t[:].rearrange("p i r w -> p (i r w)")
            dwy_flats = [t[:].rearrange("p r w -> p (r w)") for t in dwys_t]
            um_flats = [t[:].rearrange("p r w -> p (r w)") for t in umasks]
            dxs_t = [mskp.tile([128, RPU, W], f16, tag=f"dx{i}", name=f"dx{i}") for i in range(BL)]
            dx_flats = [t[:].rearrange("p r w -> p (r w)") for t in dxs_t]

            # greedy evac balancer: DVE ~1284ns, ACT ~1114ns per N=1024 op.
            # DVE starts with a handicap for its fixed non-evac load (dw build,
            # maxpools, epilogue pieces) so ScalarE takes a bigger evac share.
            ebal = {"dve": 30000.0, "act": 0.0}

            def evac(dst, src, bias):
                if ebal["dve"] + 1284 <= ebal["act"] + 1114:
                    ebal["dve"] += 1284
                    nc.vector.tensor_scalar(dst, src, bias, 0.0, OP.add, OP.max)
                else:
                    ebal["act"] += 1114
                    nc.scalar.activation(dst, src, AF.Relu, bias=bias)

            h1_of = {}  # (img, t, par) -> (h1A, h1B): A=(u=par,2+par), B=(4+par,6+par)
            h2_of = {}  # (img, t, j) -> h2 pair tile (u=2j cols 0:TS, 2j+1 cols TS:)

            def emit_l1_quad(img, t, par):
                # 4 dense K=32 MMs (4 strips, back-to-back -> concurrent) then
                # 4 accumulating dwy MMs; evacuate both 2-bank tiles.
                off = img * FHI + W + t * TS
                dwo = t * TS
                pf = p1_flat if par == 0 else p2_flat
                zA = pz1.tile([128, 2 * TS], f32, tag="z1", name=f"z1a_{img}_{t}_{par}")
                zB = pz1.tile([128, 2 * TS], f32, tag="z1", name=f"z1b_{img}_{t}_{par}")
                ztgt = [(zA, 0), (zA, TS), (zB, 0), (zB, TS)]
                for g in range(4):
                    zt, zo = ztgt[g]
                    pu = 32 * g
                    nc.tensor.matmul(
                        zt[:, zo:zo + TS], wstackA[pu:pu + 32, :],
                        pf[pu:pu + 32, off:off + TS],
                        start=True, stop=False, tile_position=(pu, 0),
                    )
                for g in range(4):
                    zt, zo = ztgt[g]
                    base = 32 * g
                    wv = w1y[base:base + 32, par * 128:(par + 1) * 128]
                    nc.tensor.matmul(
                        zt[:, zo:zo + TS], wv,
                        dwy_flats[img][base:base + 32, dwo:dwo + TS],
                        start=False, stop=True, tile_position=(base, 0),
                    )
                h1A = h1p.tile([128, 2 * TS], f16, tag="h1", name=f"h1a_{img}_{t}_{par}")
                h1B = h1p.tile([128, 2 * TS], f16, tag="h1", name=f"h1b_{img}_{t}_{par}")
                evac(h1A[:, :], zA[:, :], b1c[:])
                evac(h1B[:, :], zB[:, :], b1c[:])
                h1_of[(img, t, par)] = (h1A, h1B)

            def emit_l2_half(img, t, jj):
                # pairs (2jj, 2jj+1): u = 4jj .. 4jj+3; 4 full-mode K=128 MMs
                h1e = h1_of[(img, t, 0)]
                h1o = h1_of[(img, t, 1)]
                zG = pz2.tile([128, 2 * TS], f32, tag="z2", name=f"z2g_{img}_{t}_{jj}")
                zH = pz2.tile([128, 2 * TS], f32, tag="z2", name=f"z2h_{img}_{t}_{jj}")
                for k, zt in ((0, zG), (1, zH)):
                    j = 2 * jj + k
                    for par in range(2):
                        g = j  # u = 2j+par -> dense-group index g=j
                        src = (h1e if par == 0 else h1o)[g // 2]
                        hsl = (g % 2) * TS
                        nc.tensor.matmul(
                            zt[:, par * TS:(par + 1) * TS], w2t[:, :],
                            src[:, hsl:hsl + TS], start=True, stop=True,
                        )
                h2G = h2p.tile([128, 2 * TS], f16, tag="h2", name=f"h2g_{img}_{t}_{jj}")
                h2H = h2p.tile([128, 2 * TS], f16, tag="h2", name=f"h2h_{img}_{t}_{jj}")
                evac(h2G[:, :], zG[:, :], b2c[:])
                evac(h2H[:, :], zH[:, :], b2c[:])
                h2_of[(img, t, 2 * jj)] = h2G
                h2_of[(img, t, 2 * jj + 1)] = h2H
                if jj == 1:
                    h1_of.pop((img, t, 0))
                    h1_of.pop((img, t, 1))

            def emit_l3(img, t):
                dwo = t * TS
                z3 = pz1.tile([128, 2 * TS], f32, tag="z1", name=f"z3_{img}_{t}")
                for j in range(4):
                    # open+close each accumulation group before the next: the
                    # start=True clear is bank-granular, so interleaved groups
                    # in one bank corrupt each other
                    nc.tensor.matmul(
                        z3[32 * j:32 * j + 32, 0:TS], w3t[:, 0:32],
                        h2_of[(img, t, j)][:, 0:TS],
                        start=True, stop=False, tile_position=(0, 32 * j),
                    )
                    nc.tensor.matmul(
                        z3[32 * j:32 * j + 32, 0:TS], w3t[:, 32:64],
                        h2_of[(img, t, j)][:, TS:2 * TS],
                        start=False, stop=True, tile_position=(0, 32 * j),
                    )
                for j in range(4):
                    h2_of.pop((img, t, j))
                # dx = (z3 + b3) * umask  (dx tiles hold masked dx, NOT x_new)
                ebal["dve"] += 660
                nc.vector.scalar_tensor_tensor(
                    dx_flats[img][:, dwo:dwo + TS], z3[:, 0:TS], b3c[:],
                    um_flats[img][:, dwo:dwo + TS], OP.add, OP.mult,
                )

            # ---- epilogue: post-life 5-row stripes + life mask + store ----
            lifec_d = dramp.tile([128, 3 * W], f16)
            al5_post = strp.tile([128, 5, W], f16, tag="al5post")
            m2post = strp.tile([128, 3, W], f16, tag="m2post")
            lifec = strp.tile([128, 3 * W], f16, tag="lifec")
            # one shared out16 buffer; epilogues are sequential (tag-shared)

            dxd = dramp.tile([128, RPU * W], f16, name="dxd")
            dxag = dramp.tile([H, W], f16, name="dxag")
            hop = strp.tile([8, RPU * W], f16, tag="hop")

            def ep_A(img):
                # bounce dx to DRAM (plain full AP, cleanly ordered after stt),
                # extract alpha rows into a global [192,192] DRAM image via an
                # SBUF hop, then load the 5-row stripes with a handful of DMAs.
                nc.sync.dma_start(dxd[:, :], dx_flats[img][:, :])
                asrc = dxd[:, :].rearrange("(u c) x -> u c x", c=16)[:, 3, :]
                nc.sync.dma_start(hop[:, :], asrc)
                nc.sync.dma_start(dxag[:, :].rearrange("(u r) w -> u (r w)", r=RPU), hop[:, :])
                row_stripes(al5_post, img, dxag[:, :])
                xa5 = strp.tile([128, 5, W], f16, tag="xa5", name=f"xa5_{img}")
                row_stripes(xa5, img, x_d.ap()[img, 3, :, :])
                sl = slice(img * 64, (img + 1) * 64)
                nc.vector.tensor_add(al5_post[sl], al5_post[sl], xa5[sl])

            def ep_B(img):
                sl = slice(img * 64, (img + 1) * 64)
                stripe_pool_chain(al5_post, sl, m2post)
                nc.vector.tensor_tensor(
                    lifec[sl], m2pre[:].rearrange("p r w -> p (r w)")[sl],
                    m2post[:].rearrange("p r w -> p (r w)")[sl], OP.min,
                )
                nc.vector.tensor_scalar(lifec[sl], lifec[sl], 0.1, None, OP.is_gt)
                nc.sync.dma_start(lifec_d[sl], lifec[sl])
                life = mskp.tile([128, RPU, W], f16, tag="life", name=f"life{img}")
                for u in range(U):
                    bsrc = lifec_d[img * 64 + 8 * u: img * 64 + 8 * u + 8, :]
                    bsrc = bsrc.rearrange("s w -> (s w)").partition_broadcast(16)
                    nc.sync.dma_start(life[u * 16:(u + 1) * 16], bsrc)
                return life

            def ep_C(img, eng, life):
                o16 = bigp.tile([128, RPU, W], f16, tag="o16", name=f"o16_{img}")
                eng.tensor_add(o16[:], dxs_t[img][:], xf[:, img, 1:25, :])
                eng.tensor_mul(o16[:], o16[:], life[:])
                for u in range(U):
                    nc.sync.dma_start(
                        out_d.ap()[img, :, u * RPU:(u + 1) * RPU, :],
                        o16[u * 16:(u + 1) * 16],
                    )

            # ================= main emission =================
            # software-pipelined: step S emits L1(S) | L2(S-1) | L3(S-2)
            windows = [(0, t) for t in range(NT)] + [(1, t) for t in range(NT)]
            NW = len(windows)
            for S in range(NW + 2):
                if S < NW:
                    emit_l1_quad(*windows[S], 0)
                if 1 <= S <= NW:
                    emit_l2_half(*windows[S - 1], 0)
                if S < NW:
                    emit_l1_quad(*windows[S], 1)
                if 1 <= S <= NW:
                    emit_l2_half(*windows[S - 1], 1)
                if 2 <= S:
                    emit_l3(*windows[S - 2])
                if S == 0:
                    load_xf(1)
                    load_px(1)
                    load_um(1)
                elif S == 1:
                    emit_prelife(1)
                if 1 <= S <= 4:
                    emit_dw_chunk(1, RC * (S - 1), nc.gpsimd)
                if S == 11:
                    ep_A(0)
                elif S == 13:
                    life0 = ep_B(0)
                elif S == 15:
                    ep_C(0, nc.gpsimd, life0)
            ep_A(1)
            life1 = ep_B(1)
            ep_C(1, nc.vector, life1)

            if DBG:
                m2pre_d = nc.dram_tensor("dbg_m2pre", [128, 3 * W], f16, kind="ExternalOutput")
                m2post_d = nc.dram_tensor("dbg_m2post", [128, 3 * W], f16, kind="ExternalOutput")
                lifec_o = nc.dram_tensor("dbg_lifec", [128, 3 * W], f16, kind="ExternalOutput")
                dx0_d = nc.dram_tensor("dbg_dx0", [128, RPU * W], f16, kind="ExternalOutput")
                dwy0_d = nc.dram_tensor("dbg_dwy0", [128, RPU * W], f16, kind="ExternalOutput")
                nc.sync.dma_start(m2pre_d.ap(), m2pre[:].rearrange("p r w -> p (r w)"))
                nc.sync.dma_start(m2post_d.ap(), m2post[:].rearrange("p r w -> p (r w)"))
                nc.sync.dma_start(lifec_o.ap(), lifec[:])
                nc.sync.dma_start(dx0_d.ap(), dx_flats[0][:, :])
                nc.sync.dma_start(dwy0_d.ap(), dwy_flats[0][:, :])

    nc.compile()
    return nc


def host_prep(inputs):
    """Full inputs -> list of 8 per-core input dicts."""
    x = np.ascontiguousarray(inputs["x"], dtype=np.float32)
    fn = np.ascontiguousarray(inputs["fire_noise"], dtype=np.float32)
    w1 = np.asarray(inputs["w1"], np.float32)
    b1 = np.asarray(inputs["b1"], np.float32)
    w2 = np.asarray(inputs["w2"], np.float32)
    b2 = np.asarray(inputs["b2"], np.float32)
    w3 = np.asarray(inputs["w3"], np.float32)
    b3 = np.asarray(inputs["b3"], np.float32)

    w1a, w1b, w1c = w1[:, 0:16], w1[:, 16:32] / 8.0, w1[:, 32:48] / 8.0
    # dense K=32 L1 weights: rows 32g+0:16 = w1a.T (x), 16:32 = w1b.T (dwx)
    wstackA = np.zeros((128, 128), ml_dtypes.bfloat16)
    for g in range(4):
        wstackA[32 * g:32 * g + 16, :] = w1a.T.astype(ml_dtypes.bfloat16)
        wstackA[32 * g + 16:32 * g + 32, :] = w1b.T.astype(ml_dtypes.bfloat16)
    # dwy weights: cols 0:128 even-u (rows 32g+0:16 = w1c.T), cols 128:256 odd
    w1y = np.zeros((128, 256), ml_dtypes.bfloat16)
    for g in range(4):
        w1y[32 * g:32 * g + 16, 0:128] = w1c.T.astype(ml_dtypes.bfloat16)
        w1y[32 * g + 16:32 * g + 32, 128:256] = w1c.T.astype(ml_dtypes.bfloat16)
    w2t = w2.T.astype(ml_dtypes.bfloat16)
    w3t = np.zeros((128, 64), ml_dtypes.bfloat16)
    w3t[:, 0:16] = w3.T.astype(ml_dtypes.bfloat16)
    w3t[:, 48:64] = w3.T.astype(ml_dtypes.bfloat16)
    b3col = np.tile(b3, U).reshape(128, 1).astype(np.float32)

    shared = {
        "wstackA": wstackA, "w1y": w1y, "w2t": w2t, "w3t": w3t,
        "b1": b1.reshape(128, 1).astype(np.float32),
        "b2": b2.reshape(128, 1).astype(np.float32),
        "b3": b3col,
    }
    xh = x.astype(ml_dtypes.bfloat16)
    um = (fn[:, 0] <= 0.5).astype(ml_dtypes.bfloat16)
    in_maps = []
    for core in range(8):
        m = dict(shared)
        m["x"] = xh[2 * core:2 * core + 2]
        m["fn"] = um[2 * core:2 * core + 2]
        in_maps.append(m)
    return in_maps


_NC_CACHE = None


def kernel(**inputs):
    global _NC_CACHE
    from concourse.bass_utils import run_bass_kernel_spmd
    if _NC_CACHE is None:
        _NC_CACHE = build_nc()
    in_maps = host_prep(inputs)
    res = run_bass_kernel_spmd(_NC_CACHE, in_maps, core_ids=list(range(8)))
    return np.concatenate(
        [np.asarray(res.results[i]["out"], dtype=np.float32) for i in range(8)], axis=0
    )
